# revision 1
# baseline (speedup 1.0000x reference)
"""Trainium2 Bass kernel for DeiT-style attention + depthwise-conv block.

Computes, for x [N=32, L=577, C=768]:
  qkv = x @ w_qkv.T -> q,k,v (12 heads, hd=64)
  attn = softmax(q k^T / 8) @ v
  out  = attn (+ depthwise3x3(v) on patch tokens) @ w_proj.T + b_proj

Sharding: data-parallel over batch, 4 samples per core x 8 NeuronCores.
Host pre-transposes x / w_qkv / w_proj so no on-device input transposes
are needed; all layouts stay channel-major until the final projection,
which emits the natural [L, C] layout directly.
"""
import sys

sys.path.insert(0, "/opt/trn_rl_repo")

import numpy as np

import concourse.bacc as bacc
import concourse.mybir as mybir
import concourse.tile as tile
from concourse.bass_utils import run_bass_kernel_spmd

F32 = mybir.dt.float32
F32R = mybir.dt.float32r
BF16 = mybir.dt.bfloat16
Exp = mybir.ActivationFunctionType.Exp
MULT = mybir.AluOpType.mult
ADD = mybir.AluOpType.add

N_CORES = 8
S = 4            # samples per core
C, L, H, HD = 768, 577, 12, 64
CT = C // 128    # 6 channel tiles
KT = 3 * C // 128  # 18 qkv row tiles
SCALE = HD ** -0.5
# L split into partition chunks
L_CHUNKS = [(i * 128, min(128, L - i * 128)) for i in range((L + 127) // 128)]
# free-dim split of L for matmul N<=512 (one PSUM bank per matmul)
LN_SPLIT = [(0, 512), (512, 65)]
IMG = 24         # spatial side; L-1 == IMG*IMG
PAD = IMG + 2    # padded side

_CACHE = {}
last_results = None  # BassKernelResults of the most recent run (for test harness)


def _build_nc(repeat=1, stages="full"):
    key = (repeat, stages)
    if key in _CACHE:
        return _CACHE[key]
    nc = bacc.Bacc("TRN2", target_bir_lowering=False, debug=False,
                   num_devices=N_CORES)
    xT_d = nc.declare_dram_parameter("xT", [S, C, L], BF16, isOutput=False)
    wqkvT_d = nc.declare_dram_parameter("wqkvT", [C, 3 * C], BF16, isOutput=False)
    wprojT_d = nc.declare_dram_parameter("wprojT", [C, C], F32, isOutput=False)
    wdwc_d = nc.declare_dram_parameter("wdwc", [C, 9], F32, isOutput=False)
    bdwc_d = nc.declare_dram_parameter("bdwc", [C, 1], F32, isOutput=False)
    bproj_d = nc.declare_dram_parameter("bproj", [1, C], F32, isOutput=False)
    y_d = nc.declare_dram_parameter("y", [S, L, C], F32, isOutput=True)

    with tile.TileContext(nc) as tc:
        with tc.tile_pool(name="wpool", bufs=1) as wpool, \
             tc.tile_pool(name="work", bufs=2) as work, \
             tc.tile_pool(name="mm", bufs=2, space="PSUM") as psum_mm, \
             tc.tile_pool(name="pv", bufs=2, space="PSUM") as psum_pv:

            # ---- resident small weights (qkv weights streamed per use) ----
            wprojT = []
            for k in range(CT):
                t = wpool.tile([128, C], F32R, tag="wprojT", bufs=CT, name=f"wprojT{k}")
                nc.sync.dma_start(t[:], wprojT_d[k * 128:(k + 1) * 128, :].bitcast(F32R))
                wprojT.append(t)
            wdwc = []
            bdwc = []
            for k in range(CT):
                t = wpool.tile([128, 9], F32, tag="wdwc", bufs=CT, name=f"wdwc{k}")
                nc.sync.dma_start(t[:], wdwc_d[k * 128:(k + 1) * 128, :])
                wdwc.append(t)
                t = wpool.tile([128, 1], F32, tag="bdwc", bufs=CT, name=f"bdwc{k}")
                nc.sync.dma_start(t[:], bdwc_d[k * 128:(k + 1) * 128, :])
                bdwc.append(t)
            bproj_row = wpool.tile([1, C], F32, tag="bprow")
            nc.sync.dma_start(bproj_row[:], bproj_d[:])
            bproj_bc = wpool.tile([128, C], F32, tag="bpbc")
            nc.gpsimd.partition_broadcast(bproj_bc[:], bproj_row[:])

            import contextlib
            rep_ctx = tc.For_i(0, repeat, 1) if repeat > 1 else contextlib.nullcontext()
            with rep_ctx:
              for s in range(S):
                  # ---- load xT for this sample ----
                  xT = []
                  for k in range(CT):
                      t = work.tile([128, L], BF16, tag="xT", bufs=2 * CT, name=f"xT{k}")
                      nc.sync.dma_start(t[:], xT_d[s, k * 128:(k + 1) * 128, :])
                      xT.append(t)

                  # v-part weights [C-chunk, 768] (streamed per sample; lhsT
                  # slices for qkv m>=12 and rhs for v_LC)
                  wv = []
                  for k in range(CT):
                      t = work.tile([128, C], BF16, tag="wv", bufs=CT, name=f"wv{k}")
                      nc.sync.dma_start(
                          t[:], wqkvT_d[k * 128:(k + 1) * 128, 2 * C:3 * C])
                      wv.append(t)
                  # q,k weights: one big DMA per channel chunk
                  wqk = []
                  for k in range(CT):
                      t = work.tile([128, 2 * C], BF16, tag="wqk", bufs=CT, name=f"wqk{k}")
                      nc.sync.dma_start(
                          t[:], wqkvT_d[k * 128:(k + 1) * 128, 0:2 * C])
                      wqk.append(t)

                  # ---- QKV: qkvT[3C, L] = w_qkvT.T @ xT  (f32r) ----
                  # m tiles 0..11 -> q,k rows (bf16), 12..17 -> v rows (bf16)
                  qk_sb = []   # 12 tiles [128, L] bf16 (q: 0..5, k: 6..11)
                  v_ch = []    # 6 tiles [128, L] bf16 (channel-major v)
                  for m in range(KT):
                      p = psum_mm.tile([128, 768], F32, tag="mm")
                      for k in range(CT):
                          if m < 12:
                              w_ap = wqk[k][:, m * 128:(m + 1) * 128]
                          else:
                              w_ap = wv[k][:, (m - 12) * 128:(m - 11) * 128]
                          for (n0, nn) in ((0, 512), (512, 65)):
                              nc.tensor.matmul(
                                  p[:, n0:n0 + nn],
                                  w_ap,
                                  xT[k][:, n0:n0 + nn],
                                  start=(k == 0), stop=(k == CT - 1))
                      dst = work.tile([128, L], BF16,
                                      tag="qk" if m < 12 else "vch",
                                      bufs=24 if m < 12 else 2 * CT,
                                      name=f"qkv{m}")
                      nc.any.tensor_copy(dst[:], p[:, 0:L])
                      (qk_sb if m < 12 else v_ch).append(dst)

                  # ---- v_LC: v[L, C] = xT.T @ w_vT, stored per-head 65-wide
                  # blocks (64 cols of v_h + ones column) for the PV matmul ----
                  v65 = []
                  for (l0, lp) in L_CHUNKS:
                      t = work.tile([128, H * 65], BF16, tag="v65", bufs=8, name="v65t")
                      t3 = t[:].rearrange("p (h w) -> p h w", h=H, w=65)
                      nc.vector.memset(t3[0:lp, :, 64:65], 1.0)
                      p = psum_mm.tile([128, 768], F32, tag="mm")
                      for (n0, nn) in ((0, 512), (512, 256)):
                          for k in range(CT):
                              nc.tensor.matmul(
                                  p[0:lp, n0:n0 + nn],
                                  xT[k][:, l0:l0 + lp],
                                  wv[k][:, n0:n0 + nn],
                                  start=(k == 0), stop=(k == CT - 1))
                      nc.any.tensor_copy(
                          t3[0:lp, :, 0:64],
                          p[0:lp, 0:768].rearrange("p (h w) -> p h w", h=H, w=64))
                      v65.append(t)

                  # ---- attention per head ----
                  attn = []  # 6 tiles [128, L] f32r: normalized attn out (ch-major)
                  for ct in range(CT):
                      attn.append(work.tile([128, L], F32R, tag="attn", bufs=2 * CT, name=f"attn{ct}"))
                  def conv_prep(ct):
                      vp = work.tile([128, PAD * PAD], BF16, tag="vpad", bufs=2,
                                     name="vp")
                      vp3 = vp[:].rearrange("p (y x) -> p y x", y=PAD, x=PAD)
                      nc.vector.memset(vp[:], 0.0)
                      nc.vector.tensor_copy(
                          vp3[:, 1:1 + IMG, 1:1 + IMG],
                          v_ch[ct][:, 1:L].rearrange("p (y x) -> p y x", y=IMG, x=IMG))
                      acc = work.tile([128, IMG * IMG], BF16, tag="cacc", bufs=2,
                                      name="cacc")
                      acc3 = acc[:].rearrange("p (y x) -> p y x", y=IMG, x=IMG)

                      def tap(dy, dx):
                          return vp3[:, dy:dy + IMG, dx:dx + IMG]

                      nc.vector.tensor_scalar(
                          out=acc3, in0=tap(1, 1), scalar1=wdwc[ct][:, 4:5],
                          scalar2=None, op0=MULT)
                      for t in range(9):
                          if t == 4:
                              continue
                          tmp = work.tile([128, IMG * IMG], BF16, tag="ctmp",
                                          bufs=4, name="ctmp")
                          tmp3 = tmp[:].rearrange("p (y x) -> p y x", y=IMG, x=IMG)
                          nc.vector.tensor_scalar(
                              out=tmp3, in0=tap(t // 3, t % 3),
                              scalar1=wdwc[ct][:, t:t + 1], scalar2=None, op0=MULT)
                          nc.vector.tensor_tensor(out=acc[:], in0=acc[:],
                                                  in1=tmp[:], op=ADD)
                      return acc

                  def conv_add(ct, acc):
                      # attn[:, 1:] += acc + b_dwc
                      nc.vector.scalar_tensor_tensor(
                          out=attn[ct][:, 1:L], in0=acc[:],
                          scalar=bdwc[ct][:, 0:1],
                          in1=attn[ct][:, 1:L].bitcast(F32),
                          op0=ADD, op1=ADD)

                  conv_accs = {}
                  for h in (range(H) if stages in ("full", "noconv") else []):
                      qt = qk_sb[h // 2]
                      kt_ = qk_sb[6 + h // 2]
                      hb = (h % 2) * 64
                      if stages == "full" and h % 2 == 0:
                          conv_accs[h // 2] = conv_prep(h // 2)
                      # scoresT chunks [Lk_chunk, L] + exp -> bf16 SBUF
                      expS = []
                      for (l0, lp) in L_CHUNKS:
                          p = psum_mm.tile([128, 768], F32, tag="mm")
                          for (n0, nn) in LN_SPLIT:
                              nc.tensor.matmul(
                                  p[0:lp, n0:n0 + nn],
                                  kt_[hb:hb + 64, l0:l0 + lp],
                                  qt[hb:hb + 64, n0:n0 + nn],
                                  start=True, stop=True)
                          e = work.tile([128, L], BF16, tag="expS", bufs=6, name="expSt")
                          nc.scalar.activation(e[0:lp, :], p[0:lp, 0:L], Exp,
                                               scale=SCALE)
                          expS.append(e)
                      # PV: [65, L] accumulated over Lk chunks; row 64 = sums
                      pv = psum_pv.tile([128, L], F32, tag="pv")
                      for ci, (l0, lp) in enumerate(L_CHUNKS):
                          for (n0, nn) in LN_SPLIT:
                              nc.tensor.matmul(
                                  pv[0:65, n0:n0 + nn],
                                  v65[ci][0:lp, h * 65:(h + 1) * 65],
                                  expS[ci][0:lp, n0:n0 + nn],
                                  start=(ci == 0), stop=(ci == len(L_CHUNKS) - 1))
                      # normalize: recip of sums row (via SBUF), broadcast, multiply
                      sums = work.tile([1, L], F32, tag="sums", bufs=4, name="sums")
                      nc.scalar.copy(sums[:], pv[64:65, :])
                      rec = work.tile([1, L], F32, tag="rec", bufs=4, name="rec")
                      nc.vector.reciprocal_approx_fast(out=rec[:], in_=sums[:])
                      bc = work.tile([64, L], F32, tag="bc", bufs=3, name="bc")
                      nc.gpsimd.partition_broadcast(bc[:], rec[:])
                      nc.vector.tensor_tensor(
                          out=attn[h // 2][hb:hb + 64, :],
                          in0=pv[0:64, :], in1=bc[:], op=MULT)
                      if stages == "full" and h % 2 == 1:
                          conv_add(h // 2, conv_accs.pop(h // 2))

                  # ---- proj: y[L, C] = attn.T @ w_projT + b_proj ----
                  if stages == "qkv":
                      zsrc = work.tile([128, L], F32, tag="zsrc", bufs=1, name="zsrc")
                      nc.vector.memset(zsrc[:], 0.0)
                      for ct in range(CT):
                          nc.vector.tensor_copy(attn[ct][:], zsrc[:])
                  for (l0, lp) in L_CHUNKS:
                      p = psum_mm.tile([128, 768], F32, tag="mm")
                      for (n0, nn) in ((0, 512), (512, 256)):
                          for k in range(CT):
                              nc.tensor.matmul(
                                  p[0:lp, n0:n0 + nn],
                                  attn[k][:, l0:l0 + lp],
                                  wprojT[k][:, n0:n0 + nn],
                                  start=(k == 0), stop=(k == CT - 1))
                      ysb = work.tile([128, C], F32, tag="ysb", bufs=2)
                      nc.vector.tensor_tensor(
                          out=ysb[0:lp, :], in0=p[0:lp, :], in1=bproj_bc[0:lp, :],
                          op=ADD)
                      nc.sync.dma_start(y_d[s, l0:l0 + lp, :], ysb[0:lp, :])

    nc.compile()
    _CACHE[key] = nc
    return nc


def make_in_maps(x, w_qkv, w_proj, b_proj, w_dwc, b_dwc):
    x = np.asarray(x, dtype=np.float32)
    N = x.shape[0]
    assert N == N_CORES * S
    import ml_dtypes
    wqkvT = np.ascontiguousarray(
        np.asarray(w_qkv, np.float32).T.astype(ml_dtypes.bfloat16))    # [C, 3C]
    wprojT = np.ascontiguousarray(np.asarray(w_proj, np.float32).T)    # [C, C]
    wdwc9 = np.ascontiguousarray(np.asarray(w_dwc, np.float32).reshape(C, 9))
    bdwc = np.ascontiguousarray(np.asarray(b_dwc, np.float32).reshape(C, 1))
    bproj = np.ascontiguousarray(np.asarray(b_proj, np.float32).reshape(1, C))

    in_maps = []
    for i in range(N_CORES):
        xs = x[i * S:(i + 1) * S]                       # [S, L, C]
        xT = np.ascontiguousarray(
            xs.transpose(0, 2, 1).astype(ml_dtypes.bfloat16))  # [S, C, L]
        in_maps.append({"xT": xT, "wqkvT": wqkvT, "wprojT": wprojT,
                        "wdwc": wdwc9, "bdwc": bdwc, "bproj": bproj})
    return in_maps


def kernel(x, w_qkv, w_proj, b_proj, w_dwc, b_dwc):
    global last_results
    nc = _build_nc()
    in_maps = make_in_maps(x, w_qkv, w_proj, b_proj, w_dwc, b_dwc)
    last_results = run_bass_kernel_spmd(nc, in_maps, list(range(N_CORES)))
    y = np.concatenate([r["y"] for r in last_results.results], axis=0)
    return y.astype(np.float32)

